# revision 10
# baseline (speedup 1.0000x reference)
"""CosfacePairwiseLoss Trainium2 kernel (4 NeuronCores, Bass/Tile).

Strategy (v4):
- The amortized per-exec cost through the axon/PJRT dispatch path grows with
  device count (~1.0 ms for 1 core, ~1.2 ms for 4, ~2.0 ms for 8) and device
  compute adds on top of it, so the sweet spot is 4 cores: dispatch stays
  cheap while the GEMM + exp (the only large compute terms) still shard 4x.
- No collective: every core receives the FULL feat (replicated, bf16),
  normalizes all 8192 rows locally (1024 rows per DMA; per-DMA fixed costs
  ~1.5us make small transfers dominate otherwise), transposes via the DMA
  xbar, and quantizes to fp8e4m3 (x16). Rows are sharded 2048/core.
- Dense pass (fp8 double-row matmuls, 256-contraction per instruction):
  PSUM holds 256*sim. Host-sorted rows put positives in a 256-wide diagonal
  band; a 0/1 mask (fp8) times -8533.33 is added (DVE scalar_tensor_tensor)
  so positives underflow to exactly 0 in the ACT exp(30/256*x) row-sum.
- Band pass (positives): all offsets static — the own-rows band region
  (R+128 cols) is copied once at the pid-dependent offset, and the host
  pre-slices each row's 256-col band mask. sum(exp(9 - 30*sim)) per row
  with margin folded into constants. Per-tile results are collected and the
  softplus(ln P + ln N) chain runs once on [128, MT] at the end.
- Each core reduces its row losses to a [128,1] column; the host sums
  4*128 floats and divides by N (the only unsharded work).
"""
import numpy as np
import ml_dtypes

import concourse.bass as bass
import concourse.bacc as bacc
import concourse.mybir as mybir
import concourse.tile as tile
from concourse.bass_utils import run_bass_kernel_spmd

F32 = mybir.dt.float32
BF16 = mybir.dt.bfloat16
F8 = mybir.dt.float8e4
AF = mybir.ActivationFunctionType
ALU = mybir.AluOpType
DR = mybir.MatmulPerfMode.DoubleRow

F8SCALE = 16.0  # features scaled x16 before fp8 quantization; sim = psum/256
SIMMUL = F8SCALE * F8SCALE

N, D, NCORES = 8192, 512, 4
R = N // NCORES  # rows per core (2048)
MT = R // 128  # row-tiles per core (16)
MEGA = 1024  # rows per phase-A DMA
NMEGA = N // MEGA  # 8
MB = MEGA // 128  # row-tiles per mega tile (8)
W = 256  # band window width
PAD = 64  # fT padding each side
NP_ = N + 2 * PAD  # padded columns
GW = 1024  # psum group width (2 chunks)
NG = N // GW  # psum groups (8)
BREG = R + 2 * PAD  # band region width (own rows +-64)

_CACHED = {}


def _build_nc():
    nc = bacc.Bacc("TRN2", target_bir_lowering=False, debug=False, num_devices=NCORES)

    feat_in = nc.dram_tensor("feat_in", [N, D], BF16, kind="ExternalInput").ap()
    mask_in = nc.dram_tensor("mask_in", [R, NP_], F8, kind="ExternalInput").ap()
    bmask_in = nc.dram_tensor("bmask_in", [R, W], F8, kind="ExternalInput").ap()
    o_loss = nc.dram_tensor("o_loss", [128, 1], F32, kind="ExternalOutput").ap()

    with tile.TileContext(nc) as tc:
        with (
            tc.tile_pool(name="io", bufs=2) as io,
            tc.tile_pool(name="fbp", bufs=2) as fbp,
            tc.tile_pool(name="stats", bufs=8) as stats,
            tc.tile_pool(name="singles", bufs=1) as singles,
            tc.tile_pool(name="ftmp", bufs=2) as ftp,
            tc.tile_pool(name="maskp", bufs=2) as maskp,
            tc.tile_pool(name="bmaskp", bufs=2) as bmaskp,
            tc.tile_pool(name="up", bufs=3) as upool,
            tc.tile_pool(name="ep", bufs=3) as epool,
            tc.tile_pool(name="bsmall", bufs=2) as bsmall,
            tc.tile_pool(name="nsp", bufs=2) as nsp,
            tc.tile_pool(name="psmain", bufs=3, space="PSUM") as psmain,
            tc.tile_pool(name="psband", bufs=2, space="PSUM") as psband,
            tc.tile_pool(name="dram", bufs=1, space="DRAM") as dram,
        ):
            cc = dram.tile([N, D], BF16)  # normalized rows, core-local

            bias150 = singles.tile([128, 1], F32)
            nc.vector.memset(bias150, -150.0)

            # ---- Phase A: normalize ALL rows (1024 rows per DMA) ----
            for t in range(NMEGA):
                x = io.tile([128, MB, D], BF16, tag="x")
                nc.sync.dma_start(
                    out=x,
                    in_=feat_in[t * MEGA : (t + 1) * MEGA, :].rearrange(
                        "(b p) d -> p b d", p=128
                    ),
                )
                scr = io.tile([128, MB, D], BF16, tag="scr")  # values unused
                ss = stats.tile([128, MB], F32, tag="ss")
                for b in range(MB):
                    nc.scalar.activation(
                        scr[:, b, :], x[:, b, :], AF.Square,
                        accum_out=ss[:, b : b + 1],
                    )
                ssc = stats.tile([128, MB], F32, tag="ssc")
                nc.vector.tensor_scalar_max(ssc, ss, 1e-16)
                lnss = stats.tile([128, MB], F32, tag="lnss")
                nc.scalar.activation(lnss, ssc, AF.Ln)
                rinv = stats.tile([128, MB], F32, tag="rinv")
                nc.scalar.activation(rinv, lnss, AF.Exp, scale=-0.5)
                fb = fbp.tile([128, MB, D], BF16, tag="fb")
                for b in range(MB):
                    nc.vector.tensor_scalar_mul(
                        fb[:, b, :], x[:, b, :], rinv[:, b : b + 1]
                    )
                nc.sync.dma_start(
                    out=cc[t * MEGA : (t + 1) * MEGA, :].rearrange(
                        "(b p) d -> p b d", p=128
                    ),
                    in_=fb,
                )

            pid_pe = nc.tensor.partition_id()
            pid_dve = nc.vector.partition_id()

            # ---- Phase B: transpose each k-slice, quantize to fp8 (x16) ----
            ft8 = singles.tile([128, 4, NP_], F8, name="ft8")
            ft8_own = singles.tile([128, 4, R], F8, name="ft8_own")
            ft8_band = singles.tile([128, 4, BREG], F8, name="ft8_band")
            for k in range(4):
                nc.vector.memset(ft8[:, k, 0:PAD], 0.0)
                nc.vector.memset(ft8[:, k, NP_ - PAD : NP_], 0.0)
                ftmp = ftp.tile([128, N], BF16, tag="ftmp")
                nc.sync.dma_start_transpose(out=ftmp, in_=cc[:, bass.ts(k, 128)])
                nc.vector.tensor_scalar_mul(ft8[:, k, PAD : PAD + N], ftmp, F8SCALE)
                nc.vector.tensor_scalar_mul(
                    ft8_own[:, k, :], ftmp[:, bass.ds(pid_dve * R, R)], F8SCALE
                )
                # band region: own rows +-PAD, from the padded ft8 so the
                # edges of core 0 / core 3 read zero-pads
                nc.vector.tensor_copy(
                    ft8_band[:, k, :], ft8[:, k, bass.ds(pid_dve * R, BREG)]
                )

            # ---- Phase C: per row-tile ----
            pcols = singles.tile([128, MT], F32)
            ncols = singles.tile([128, MT], F32)
            for m in range(MT):
                mt_t = maskp.tile([128, NP_], F8, tag="mask")
                nc.sync.dma_start(out=mt_t, in_=mask_in[bass.ts(m, 128), :])
                bm_t = bmaskp.tile([128, W], F8, tag="bmask")
                nc.sync.dma_start(out=bm_t, in_=bmask_in[bass.ts(m, 128), :])
                nsum = nsp.tile([128, NG], F32, tag="nsum")

                for g in range(NG):
                    ps = psmain.tile([128, GW], F32, tag="ps")
                    for ks in range(2):
                        for half in range(2):
                            nc.tensor.matmul(
                                ps[:, bass.ts(half, 512)],
                                ft8_own[:, 2 * ks : 2 * ks + 2, bass.ts(m, 128)],
                                ft8[
                                    :,
                                    2 * ks : 2 * ks + 2,
                                    PAD + GW * g + 512 * half : PAD
                                    + GW * g
                                    + 512 * (half + 1),
                                ],
                                start=(ks == 0),
                                stop=(ks == 1),
                                perf_mode=DR,
                            )
                    u = upool.tile([128, GW], F32, tag="u")
                    nc.vector.scalar_tensor_tensor(
                        u,
                        in0=mt_t[:, PAD + GW * g : PAD + GW * (g + 1)],
                        scalar=-33.333333 * SIMMUL,
                        in1=ps,
                        op0=ALU.mult,
                        op1=ALU.add,
                    )
                    e = epool.tile([128, GW], BF16, tag="e")
                    nc.scalar.activation(
                        e, u, AF.Exp, scale=30.0 / SIMMUL,
                        accum_out=nsum[:, g : g + 1],
                    )

                # band (positives) pass — all offsets static: window of
                # global tile g starts at own-relative col 128*m - 64, i.e.
                # ft8_band col 128*m
                bp = psband.tile([128, W], F32, tag="bps")
                for k in range(4):
                    nc.tensor.matmul(
                        bp,
                        ft8_own[:, k, bass.ts(m, 128)],
                        ft8_band[:, k, 128 * m : 128 * m + W],
                        start=(k == 0),
                        stop=(k == 3),
                    )
                ub = bsmall.tile([128, W], F32, tag="ub")
                nc.vector.scalar_tensor_tensor(
                    ub,
                    in0=bm_t,
                    scalar=5.3 * SIMMUL,
                    in1=bp,
                    op0=ALU.mult,
                    op1=ALU.subtract,
                )
                eb = bsmall.tile([128, W], F32, tag="eb")
                nc.scalar.activation(
                    eb, ub, AF.Exp, scale=30.0 / SIMMUL, bias=bias150,
                    accum_out=pcols[:, m : m + 1],
                )
                nc.vector.reduce_sum(
                    ncols[:, m : m + 1], nsum, axis=mybir.AxisListType.X
                )

            # ---- combine (batched): loss = softplus(ln P + ln N) ----
            lp = singles.tile([128, MT], F32)
            nc.scalar.activation(lp, pcols, AF.Ln)
            lnn = singles.tile([128, MT], F32)
            nc.scalar.activation(lnn, ncols, AF.Ln)
            xr = singles.tile([128, MT], F32)
            nc.vector.tensor_tensor(xr, lp, lnn, op=ALU.add)
            er = singles.tile([128, MT], F32)
            nc.scalar.activation(er, xr, AF.Exp)
            er1 = singles.tile([128, MT], F32)
            nc.vector.tensor_scalar_add(er1, er, 1.0)
            losses = singles.tile([128, MT], F32)
            nc.scalar.activation(losses, er1, AF.Ln)

            lsum = singles.tile([128, 1], F32)
            nc.vector.reduce_sum(lsum, losses, axis=mybir.AxisListType.X)
            nc.sync.dma_start(out=o_loss, in_=lsum)

    nc.compile()
    return nc


def _prep_inputs(feat: np.ndarray, label: np.ndarray):
    """Sort rows by label, cast to bf16, build per-core masks."""
    perm = np.argsort(label, kind="stable")
    lab64 = np.asarray(label)[perm].astype(np.int64)
    feat_s = np.ascontiguousarray(
        np.asarray(feat, dtype=np.float32)[perm]
    ).astype(ml_dtypes.bfloat16)

    # verify every row's group fits its tile's band window
    starts = np.searchsorted(lab64, lab64, side="left")
    ends = np.searchsorted(lab64, lab64, side="right")
    rows = np.arange(N)
    woff = (rows // 128) * 128 - PAD  # window [woff, woff + W)
    assert (starts >= woff).all() and (ends <= woff + W).all(), (
        "label group exceeds band window; widen W"
    )

    mask_full = lab64[:, None] == lab64[None, :]
    in_maps = []
    for c in range(NCORES):
        sl = slice(c * R, (c + 1) * R)
        maskp = np.zeros((R, NP_), dtype=ml_dtypes.float8_e4m3fn)
        maskp[:, PAD : PAD + N] = mask_full[sl].astype(ml_dtypes.float8_e4m3fn)
        # band mask: row r (global) sees window cols [woff(r), woff(r)+W)
        bmask = np.zeros((R, W), dtype=ml_dtypes.float8_e4m3fn)
        for j in range(MT):
            g = c * MT + j
            lo = 128 * g - PAD
            rsl = slice(j * 128, (j + 1) * 128)
            csl = slice(max(lo, 0), min(lo + W, N))
            out0 = max(lo, 0) - lo
            bmask[rsl, out0 : out0 + csl.stop - csl.start] = mask_full[
                sl, :
            ][rsl, csl].astype(ml_dtypes.float8_e4m3fn)
        in_maps.append(
            {"feat_in": feat_s, "mask_in": maskp, "bmask_in": bmask}
        )
    return in_maps


def kernel(feat: np.ndarray, label: np.ndarray) -> np.ndarray:
    feat = np.asarray(feat, dtype=np.float32)
    label = np.asarray(label)
    assert feat.shape == (N, D) and label.shape == (N,)

    in_maps = _prep_inputs(feat, label)

    if "nc" not in _CACHED:
        _CACHED["nc"] = _build_nc()
    nc = _CACHED["nc"]

    res = run_bass_kernel_spmd(nc, in_maps, core_ids=list(range(NCORES)))
    total = sum(float(res.results[c]["o_loss"].sum()) for c in range(NCORES))
    return np.float32(total / N)
